# revision 25
# baseline (speedup 1.0000x reference)
"""DeepSeek-V3 MoE block on 8 trn2 NeuronCores.

Expert-parallel sparse MoE, fp16 datapath:
  - host computes routing (top-k indices AND combine weights) in fp32 numpy;
    the device never sees the gate -- it receives gathered tokens, fp16
    weights, and a per-token fp32 scale applied at the down projection
  - experts sorted by token count into 4 tiers of 8; core c slot k holds the
    (8k+c)-th busiest expert; slot capacity = exact max count of its tier so
    every core runs the identical program (SPMD) with zero weight duplication
  - all matmul operands fp16 (1 cycle/row on the PE, half the HBM traffic of
    fp32); accumulation in fp32 PSUM; outputs shipped back fp16
  - weights/tokens are host-pretransposed to partition-major layouts so every
    load is a handful of fully-contiguous DMAs
  - compute order: smallest slot first (tiny DMA prefix starts the PE early),
    then the shared expert (intermediate dim sharded 8-way; its compute
    covers the big slots' weight stream), then remaining slots descending
  - host sums the 8 shared-expert partials and scatter-adds the routed rows
"""

import os
import sys

sys.path.insert(0, "/opt/trn_rl_repo")

import numpy as np

import concourse.bacc as bacc
import concourse.bass as bass
import concourse.mybir as mybir
import concourse.tile as tile
from concourse.bass_utils import run_bass_kernel_spmd

F32 = mybir.dt.float32
F16 = mybir.dt.float16
AF = mybir.ActivationFunctionType
ALU = mybir.AluOpType
AX = mybir.AxisListType

T, H, I, IS, E = 1024, 1024, 512, 2048, 32
G, TOPK_GROUP, TOP_K = 8, 4, 8
SCALE = 2.5
NCORES = 8
S = E // NCORES          # expert slots per core
ISH = IS // NCORES       # shared-expert intermediate shard
P128 = 128
HT = H // P128           # 8 h-tiles
IT = I // P128           # 4 i-tiles
IST = ISH // P128        # 2 shared i-tiles

LAST_RESULTS = None      # BassKernelResults of the most recent run


def _install_ntff_hook():
    """Provide antenv.axon_hooks + the ctypes NTFF profile hook when the
    container image lacks them (needed only for trace=True)."""
    import contextlib
    import ctypes
    import types

    try:
        from antenv.axon_hooks import get_axon_ntff_profile_hook  # noqa: F401
        return True
    except ImportError:
        pass
    try:
        import antenv
        so_path = "/opt/axon/libaxon_pjrt.so"
        lib = ctypes.CDLL(so_path)
        if not hasattr(lib, "axon_start_nrt_profile"):
            return False
        lib.axon_start_nrt_profile.argtypes = [
            ctypes.POINTER(ctypes.c_int64), ctypes.c_size_t]
        lib.axon_start_nrt_profile.restype = ctypes.c_int64
        lib.axon_stop_nrt_profile.argtypes = [ctypes.c_char_p]
        lib.axon_stop_nrt_profile.restype = ctypes.c_int64

        @contextlib.contextmanager
        def _hook(output_dir, device_ids):
            import jax
            jax.devices()
            if device_ids:
                ids = (ctypes.c_int64 * len(device_ids))(*device_ids)
                rc = lib.axon_start_nrt_profile(ids, len(device_ids))
            else:
                rc = lib.axon_start_nrt_profile(None, 0)
            if rc != 0:
                raise RuntimeError(f"axon_start_nrt_profile rc={rc}")
            try:
                yield
            finally:
                n = lib.axon_stop_nrt_profile(str(output_dir).encode())
                print(f"ntff profile: {n} file(s) -> {output_dir}",
                      file=sys.stderr)

        mod = types.ModuleType("antenv.axon_hooks")
        _state = {"hook": _hook}
        mod.set_axon_ntff_profile_hook = lambda h: _state.__setitem__("hook", h)
        mod.get_axon_ntff_profile_hook = lambda: _state["hook"]
        sys.modules["antenv.axon_hooks"] = mod
        antenv.axon_hooks = mod
        return True
    except Exception:
        return False


def _host_routing(x, gate_w, e_bias):
    """fp32 numpy mirror of reference._routing_combine.

    Returns (emask [T,E] bool, combine [T,E] fp32)."""
    logits = x.astype(np.float32) @ gate_w.T.astype(np.float32)
    scores = 1.0 / (1.0 + np.exp(-logits))
    swb = scores + e_bias[None, :]
    swb_g = swb.reshape(T, G, E // G)
    gs = np.sort(swb_g, axis=-1)[..., -2:].sum(-1)          # top-2 sum per group
    thr4 = np.sort(gs, axis=-1)[:, -TOPK_GROUP][:, None]
    gmask = (gs >= thr4).astype(np.float32)
    smask = np.repeat(gmask, E // G, axis=-1)
    masked = swb * smask
    thr8 = np.sort(masked, axis=-1)[:, -TOP_K][:, None]
    emask = masked >= thr8
    sc = scores * emask
    combine = sc / (sc.sum(-1, keepdims=True) + 1e-20) * SCALE
    return emask, combine.astype(np.float32)


def _chunks(p, limit=512):
    """Split width p into chunks <= limit."""
    out = []
    o = 0
    while o < p:
        w = min(limit, p - o)
        out.append((o, w))
        o += w
    return out


def _make_tiers(counts):
    """Choose slot capacities + (expert, tok_offset, tok_len) piece assignment.

    Experts heavier than a threshold theta are split into near-equal pieces
    (duplicating that expert's weight DMA on the extra slots); pieces are
    sorted descending and striped into tiers of NCORES so the shared SPMD
    capacity of each tier hugs its pieces.  theta is picked by a small sweep
    minimizing max(PE-roofline, DMA-roofline)."""
    best = None
    for theta in (2048, 512, 420, 350, 300, 256):
        pieces = []
        for e in range(E):
            cnt = int(counts[e])
            if cnt == 0:
                continue
            m = max(1, -(-cnt // theta))
            base, rem = divmod(cnt, m)
            off = 0
            for j in range(m):
                ln = base + (1 if j < rem else 0)
                pieces.append((e, off, ln))
                off += ln
        S_ = -(-len(pieces) // NCORES)
        pieces.sort(key=lambda p: -p[2])
        pieces += [(-1, 0, 0)] * (S_ * NCORES - len(pieces))
        tiers = [pieces[k * NCORES:(k + 1) * NCORES] for k in range(S_)]
        P = [max(8, max(p[2] for p in tier)) for tier in tiers]
        pe_cyc = (sum(P) * 64 + sum(-(-pk // P128) for pk in P) * P128 * 32
                  + 49200)
        dma_b = len(P) * 3.15e6 + sum(P) * 2 * H + 3.7e6
        cost = max(pe_cyc / 2.4 * 1.075, dma_b / 400.0 + 4000)
        if best is None or cost < best[0]:
            best = (cost, tiers, P)
    return best[1], best[2]


def _pmajor(a, p=P128):
    """[k*128, n] -> contiguous [128, k, n] (partition-major for 1-shot DMA)."""
    k = a.shape[0] // p
    return np.ascontiguousarray(a.reshape(k, p, a.shape[1]).transpose(1, 0, 2))


def _iimajor(a):
    """[HT*128, IT*128] weight -> contiguous [128, IT, HT, 128] so each
    [:, ii] slice is one fully-contiguous DMA (per-ii streaming)."""
    m = a.shape[1] // P128
    b = a.reshape(HT, P128, m, P128).transpose(1, 2, 0, 3)
    return np.ascontiguousarray(b)


def _build_program(P):
    """Emit the SPMD Bass program for slot capacities P (list of SL ints)."""
    SL = len(P)
    nc = bacc.Bacc(target_bir_lowering=False, debug=False)

    # ---- DRAM parameters (per-core data arrives via in_maps) ----
    xe_d = [nc.dram_tensor(f"xe{k}", [P128, HT, P[k]], F16, kind="ExternalInput")
            for k in range(SL)]
    wg_d = [nc.dram_tensor(f"wg{k}", [P128, IT, HT, P128], F16,
                           kind="ExternalInput") for k in range(SL)]
    wu_d = [nc.dram_tensor(f"wu{k}", [P128, IT, HT, P128], F16,
                           kind="ExternalInput") for k in range(SL)]
    wd_d = [nc.dram_tensor(f"wd{k}", [P128, IT, H], F16, kind="ExternalInput")
            for k in range(SL)]
    nck = [(P[k] + P128 - 1) // P128 for k in range(SL)]
    sco = [sum(nck[:k]) for k in range(SL + 1)]
    sc_d = nc.dram_tensor("sc", [P128, sco[SL]], F32, kind="ExternalInput")
    xt_d = nc.dram_tensor("xt", [P128, T // 512, HT, 512], F16,
                          kind="ExternalInput")
    wsg_d = nc.dram_tensor("wsg", [P128, IST, HT, P128], F16,
                           kind="ExternalInput")
    wsu_d = nc.dram_tensor("wsu", [P128, IST, HT, P128], F16,
                           kind="ExternalInput")
    wsd_d = nc.dram_tensor("wsd", [P128, IST, H], F16, kind="ExternalInput")
    ro_d = [nc.dram_tensor(f"ro{k}", [P[k], H], F16, kind="ExternalOutput")
            for k in range(SL)]
    so_d = nc.dram_tensor("so", [T, H], F16, kind="ExternalOutput")

    with tile.TileContext(nc) as tc:
        with (
            tc.tile_pool(name="const", bufs=1) as cpool,
            tc.tile_pool(name="x", bufs=3) as xpool,
            tc.tile_pool(name="w", bufs=3) as wpool,
            tc.tile_pool(name="acts", bufs=2) as apool,
            tc.tile_pool(name="stage", bufs=3) as stpool,
            tc.tile_pool(name="ps", bufs=2, space="PSUM") as ps,
        ):
            # ---- loads, in consumption order ----
            sct = cpool.tile([P128, sco[SL]], F32, tag="sc")
            sc_t = [sct[:, sco[k]:sco[k + 1]] for k in range(SL)]

            xe_t, wg_t, wu_t, wd_t = {}, {}, {}, {}

            def _load_gu(k):
                xe_t[k] = xpool.tile([P128, HT, P[k]], F16, tag="xe", bufs=4,
                                     name=f"xe_t{k}")
                nc.sync.dma_start(out=xe_t[k][:], in_=xe_d[k][:])
                wg_t[k] = wpool.tile([P128, IT, HT, P128], F16, tag="wg",
                                     bufs=4, name=f"wg_t{k}")
                wu_t[k] = wpool.tile([P128, IT, HT, P128], F16, tag="wu",
                                     bufs=4, name=f"wu_t{k}")
                for ii in range(IT):
                    nc.sync.dma_start(out=wg_t[k][:, ii], in_=wg_d[k][:, ii])
                    nc.sync.dma_start(out=wu_t[k][:, ii], in_=wu_d[k][:, ii])

            def _load_wd(k):
                wd_t[k] = wpool.tile([P128, IT, H], F16, tag="wd", bufs=3,
                                     name=f"wd_t{k}")
                nc.sync.dma_start(out=wd_t[k][:], in_=wd_d[k][:])

            acts_t = {}

            def _gu_chain(k, mo, mw, ii):
                h1 = ps.tile([P128, 512], F32, tag="h1", name="h1")
                h2 = ps.tile([P128, 512], F32, tag="h2", name="h2")
                for h in range(HT):
                    nc.tensor.matmul(
                        h1[:, :mw], lhsT=wg_t[k][:, ii, h, :],
                        rhs=xe_t[k][:, h, mo:mo + mw],
                        start=(h == 0), stop=(h == HT - 1))
                for h in range(HT):
                    nc.tensor.matmul(
                        h2[:, :mw], lhsT=wu_t[k][:, ii, h, :],
                        rhs=xe_t[k][:, h, mo:mo + mw],
                        start=(h == 0), stop=(h == HT - 1))
                sl = stpool.tile([P128, 512], F32, tag="silu", bufs=3,
                                 name="sl")
                nc.scalar.activation(sl[:, :mw], h1[:, :mw], AF.Silu)
                nc.vector.tensor_mul(acts_t[k][ii][:, mo:mo + mw],
                                     sl[:, :mw], h2[:, :mw])

            def _slot_gu_chains(k):
                acts_t[k] = [apool.tile([P128, P[k]], F16, tag="acts", bufs=8,
                                        name=f"acts{ii}") for ii in range(IT)]
                return [(lambda k=k, mo=mo, mw=mw, ii=ii:
                         _gu_chain(k, mo, mw, ii))
                        for (mo, mw) in _chunks(P[k]) for ii in range(IT)]

            def _down_chain(k, cc):
                pp = min(P128, P[k] - cc * P128)
                ost = stpool.tile([P128, H], F16, tag="ost", bufs=4,
                                  name="ost")
                for hh in range(2):
                    dps = ps.tile([P128, H // 2], F32, tag="dps", bufs=4,
                                  name="dps")
                    for ii in range(IT):
                        nc.tensor.matmul(
                            dps[:pp, :],
                            lhsT=acts_t[k][ii][:, cc * P128:cc * P128 + pp],
                            rhs=wd_t[k][:, ii,
                                        hh * (H // 2):(hh + 1) * (H // 2)],
                            start=(ii == 0), stop=(ii == IT - 1))
                    nc.vector.tensor_scalar_mul(
                        ost[:pp, hh * (H // 2):(hh + 1) * (H // 2)],
                        dps[:pp, :], sc_t[k][:pp, cc:cc + 1])
                nc.gpsimd.dma_start(
                    out=ro_d[k][cc * P128:cc * P128 + pp, :],
                    in_=ost[:pp, :])

            def _slot_down_chains(k):
                return [(lambda k=k, cc=cc: _down_chain(k, cc))
                        for cc in range(nck[k])]

            def _merge(down, gu):
                """Emit down chains (short, DVE-paced) interleaved with the
                next phase's gate/up chains (long, PE-only): down leads, gu
                spread through the tail so the PE never idles on the DVE."""
                items = ([((i + 0.5) / len(down), f) for i, f in
                          enumerate(down)] +
                         [((j + 0.85) / len(gu), f) for j, f in
                          enumerate(gu)])
                for _, f in sorted(items, key=lambda x: x[0]):
                    f()

            # Load stream in consumption order; every tensor is ordered to
            # land a few us before its consuming phase reaches it.
            # first phase: a mid-sized slot whose compute covers the shared
            # loads; remaining slots descending with the smallest last (its
            # tiny final store shortens the drain)
            first = SL - 3
            rest = [k for k in range(SL) if k != first]

            _load_gu(first)
            _load_wd(first)
            nc.sync.dma_start(out=sct[:], in_=sc_d[:])

            xt_t = cpool.tile([P128, T // 512, HT, 512], F16, tag="xt")
            nc.sync.dma_start(out=xt_t[:, 0], in_=xt_d[:, 0])
            wsg_t = cpool.tile([P128, IST, HT, P128], F16, tag="wsg")
            wsu_t = cpool.tile([P128, IST, HT, P128], F16, tag="wsu")
            for ii in range(IST):
                nc.sync.dma_start(out=wsg_t[:, ii], in_=wsg_d[:, ii])
                nc.sync.dma_start(out=wsu_t[:, ii], in_=wsu_d[:, ii])
            nc.sync.dma_start(out=xt_t[:, 1], in_=xt_d[:, 1])
            wsd_t = cpool.tile([P128, IST, H], F16, tag="wsd")
            nc.sync.dma_start(out=wsd_t[:], in_=wsd_d[:])
            _load_gu(rest[0])
            _load_wd(rest[0])
            _load_gu(rest[1])
            _load_wd(rest[1])

            for f in _slot_gu_chains(first):
                f()
            for f in _slot_down_chains(first):
                f()

            # shared expert (intermediate shard ISH=256)
            acts_s = [apool.tile([P128, T], F16, tag="acts_s", bufs=2,
                                 name=f"acts_s{ii}") for ii in range(IST)]

            def _shared_gu_chain(mo, mw, ii):
                h1 = ps.tile([P128, 512], F32, tag="h1", name="h1")
                h2 = ps.tile([P128, 512], F32, tag="h2", name="h2")
                ci = mo // 512
                for h in range(HT):
                    nc.tensor.matmul(
                        h1[:, :mw], lhsT=wsg_t[:, ii, h, :],
                        rhs=xt_t[:, ci, h, :mw],
                        start=(h == 0), stop=(h == HT - 1))
                for h in range(HT):
                    nc.tensor.matmul(
                        h2[:, :mw], lhsT=wsu_t[:, ii, h, :],
                        rhs=xt_t[:, ci, h, :mw],
                        start=(h == 0), stop=(h == HT - 1))
                sl = stpool.tile([P128, 512], F32, tag="silu", bufs=3,
                                 name="sl")
                nc.scalar.activation(sl[:, :mw], h1[:, :mw], AF.Silu)
                nc.vector.tensor_mul(acts_s[ii][:, mo:mo + mw],
                                     sl[:, :mw], h2[:, :mw])

            for (mo, mw) in _chunks(T):
                for ii in range(IST):
                    _shared_gu_chain(mo, mw, ii)

            def _shared_down_chain(cc):
                ost = stpool.tile([P128, H], F16, tag="ost", bufs=4,
                                  name="ost")
                for hh in range(2):
                    dps = ps.tile([P128, H // 2], F32, tag="dps", bufs=4,
                                  name="dps")
                    for ii in range(IST):
                        nc.tensor.matmul(
                            dps[:],
                            lhsT=acts_s[ii][:, cc * P128:(cc + 1) * P128],
                            rhs=wsd_t[:, ii, hh * (H // 2):(hh + 1) * (H // 2)],
                            start=(ii == 0), stop=(ii == IST - 1))
                    nc.vector.tensor_copy(
                        ost[:, hh * (H // 2):(hh + 1) * (H // 2)], dps[:])
                nc.gpsimd.dma_start(
                    out=so_d[cc * P128:(cc + 1) * P128, :], in_=ost[:])

            # pipelined tail: down chains of each phase interleave with the
            # next slot's gate/up chains so the PE never idles on the DVE
            down_prev = [(lambda cc=cc: _shared_down_chain(cc))
                         for cc in range(T // P128)]
            for j, k in enumerate(rest):
                if j + 2 < len(rest):
                    _load_gu(rest[j + 2])
                    _load_wd(rest[j + 2])
                _merge(down_prev, _slot_gu_chains(k))
                down_prev = _slot_down_chains(k)
            for f in down_prev:
                f()

    nc.compile()
    return nc


def _prepare(inputs):
    """Host-side dispatch prep: returns (in_maps, P, slot_expert, tok_lists)."""
    x = np.ascontiguousarray(inputs["hidden_states"], dtype=np.float32)
    gate_w = np.asarray(inputs["gate_w"], dtype=np.float32)
    e_bias = np.asarray(inputs["e_bias"], dtype=np.float32)
    w_gate = np.asarray(inputs["w_gate"], dtype=np.float32)
    w_up = np.asarray(inputs["w_up"], dtype=np.float32)
    w_down = np.asarray(inputs["w_down"], dtype=np.float32)
    ws_gate = np.asarray(inputs["ws_gate"], dtype=np.float32)
    ws_up = np.asarray(inputs["ws_up"], dtype=np.float32)
    ws_down = np.asarray(inputs["ws_down"], dtype=np.float32)

    # ---- dispatch metadata ----
    emask, combine = _host_routing(x, gate_w, e_bias)
    counts = emask.sum(0).astype(np.int64)
    tok_lists = [np.nonzero(emask[:, e])[0] for e in range(E)]
    tiers, P = _make_tiers(counts)

    x16 = x.astype(np.float16)
    xtf = _pmajor(np.ascontiguousarray(x16.T))         # [128, HT, T]
    xt = np.ascontiguousarray(
        xtf.reshape(P128, HT, T // 512, 512).transpose(0, 2, 1, 3))
    in_maps = []
    slot_toks = []                                     # [core][slot] -> toks
    wg16 = w_gate.astype(np.float16)
    wu16 = w_up.astype(np.float16)
    wd16 = w_down.astype(np.float16)
    wgp = {}
    for c in range(NCORES):
        m = {"xt": xt,
             "wsg": _iimajor(ws_gate[:, c * ISH:(c + 1) * ISH].astype(np.float16)),
             "wsu": _iimajor(ws_up[:, c * ISH:(c + 1) * ISH].astype(np.float16)),
             "wsd": _pmajor(ws_down[c * ISH:(c + 1) * ISH, :].astype(np.float16))}
        st = []
        scs = []
        for k in range(len(P)):
            e, off, ln = tiers[k][c]
            toks = (tok_lists[e][off:off + ln] if e >= 0
                    else np.zeros(0, dtype=np.int64))
            st.append(toks)
            n = len(toks)
            n_c = (P[k] + P128 - 1) // P128
            xe = np.zeros((P128, HT, P[k]), dtype=np.float16)
            if n:
                xe[:, :, :n] = _pmajor(np.ascontiguousarray(x16[toks].T))
            scv = np.zeros(n_c * P128, dtype=np.float32)
            if n:
                scv[:n] = combine[toks, e]
            scs.append(np.ascontiguousarray(scv.reshape(n_c, P128).T))
            if e not in wgp:
                wgp[e] = ((_iimajor(wg16[e]), _iimajor(wu16[e]),
                           _pmajor(wd16[e])) if e >= 0 else
                          (np.zeros((P128, IT, HT, P128), np.float16),
                           np.zeros((P128, IT, HT, P128), np.float16),
                           np.zeros((P128, IT, H), np.float16)))
            m[f"xe{k}"] = xe
            m[f"wg{k}"], m[f"wu{k}"], m[f"wd{k}"] = wgp[e]
        m["sc"] = np.concatenate(scs, axis=1)
        slot_toks.append(st)
        in_maps.append(m)

    return in_maps, P, slot_toks


def _recombine(results, slot_toks):
    out = np.zeros((T, H), dtype=np.float32)
    for c in range(NCORES):
        out += results[c]["so"].astype(np.float32)
    for c in range(NCORES):
        for k, toks in enumerate(slot_toks[c]):
            if len(toks):
                out[toks] += results[c][f"ro{k}"][:len(toks)].astype(np.float32)
    return out


def kernel(**inputs):
    global LAST_RESULTS
    in_maps, P, slot_toks = _prepare(inputs)
    nc = _build_program(P)
    trace = bool(int(os.environ.get("KERNEL_TRACE", "0")))
    if trace:
        trace = _install_ntff_hook()
    LAST_RESULTS = run_bass_kernel_spmd(
        nc, in_maps, list(range(NCORES)), trace=trace)
    results = LAST_RESULTS.results
    return _recombine(results, slot_toks)


# revision 26
# speedup vs baseline: 1.0399x; 1.0399x over previous
"""DeepSeek-V3 MoE block on 8 trn2 NeuronCores.

Expert-parallel sparse MoE, fp16 datapath:
  - host computes routing (top-k indices AND combine weights) in fp32 numpy;
    the device never sees the gate -- it receives gathered tokens, fp16
    weights, and a per-token fp32 scale applied at the down projection
  - experts sorted by token count into 4 tiers of 8; core c slot k holds the
    (8k+c)-th busiest expert; slot capacity = exact max count of its tier so
    every core runs the identical program (SPMD) with zero weight duplication
  - all matmul operands fp16 (1 cycle/row on the PE, half the HBM traffic of
    fp32); accumulation in fp32 PSUM; outputs shipped back fp16
  - weights/tokens are host-pretransposed to partition-major layouts so every
    load is a handful of fully-contiguous DMAs
  - compute order: smallest slot first (tiny DMA prefix starts the PE early),
    then the shared expert (intermediate dim sharded 8-way; its compute
    covers the big slots' weight stream), then remaining slots descending
  - host sums the 8 shared-expert partials and scatter-adds the routed rows
"""

import os
import sys

sys.path.insert(0, "/opt/trn_rl_repo")

import numpy as np

import concourse.bacc as bacc
import concourse.bass as bass
import concourse.mybir as mybir
import concourse.tile as tile
from concourse.bass_utils import run_bass_kernel_spmd

F32 = mybir.dt.float32
F16 = mybir.dt.float16
AF = mybir.ActivationFunctionType
ALU = mybir.AluOpType
AX = mybir.AxisListType

T, H, I, IS, E = 1024, 1024, 512, 2048, 32
G, TOPK_GROUP, TOP_K = 8, 4, 8
SCALE = 2.5
NCORES = 8
S = E // NCORES          # expert slots per core
ISH = IS // NCORES       # shared-expert intermediate shard
P128 = 128
HT = H // P128           # 8 h-tiles
IT = I // P128           # 4 i-tiles
IST = ISH // P128        # 2 shared i-tiles

LAST_RESULTS = None      # BassKernelResults of the most recent run


def _install_ntff_hook():
    """Provide antenv.axon_hooks + the ctypes NTFF profile hook when the
    container image lacks them (needed only for trace=True)."""
    import contextlib
    import ctypes
    import types

    try:
        from antenv.axon_hooks import get_axon_ntff_profile_hook  # noqa: F401
        return True
    except ImportError:
        pass
    try:
        import antenv
        so_path = "/opt/axon/libaxon_pjrt.so"
        lib = ctypes.CDLL(so_path)
        if not hasattr(lib, "axon_start_nrt_profile"):
            return False
        lib.axon_start_nrt_profile.argtypes = [
            ctypes.POINTER(ctypes.c_int64), ctypes.c_size_t]
        lib.axon_start_nrt_profile.restype = ctypes.c_int64
        lib.axon_stop_nrt_profile.argtypes = [ctypes.c_char_p]
        lib.axon_stop_nrt_profile.restype = ctypes.c_int64

        @contextlib.contextmanager
        def _hook(output_dir, device_ids):
            import jax
            jax.devices()
            if device_ids:
                ids = (ctypes.c_int64 * len(device_ids))(*device_ids)
                rc = lib.axon_start_nrt_profile(ids, len(device_ids))
            else:
                rc = lib.axon_start_nrt_profile(None, 0)
            if rc != 0:
                raise RuntimeError(f"axon_start_nrt_profile rc={rc}")
            try:
                yield
            finally:
                n = lib.axon_stop_nrt_profile(str(output_dir).encode())
                print(f"ntff profile: {n} file(s) -> {output_dir}",
                      file=sys.stderr)

        mod = types.ModuleType("antenv.axon_hooks")
        _state = {"hook": _hook}
        mod.set_axon_ntff_profile_hook = lambda h: _state.__setitem__("hook", h)
        mod.get_axon_ntff_profile_hook = lambda: _state["hook"]
        sys.modules["antenv.axon_hooks"] = mod
        antenv.axon_hooks = mod
        return True
    except Exception:
        return False


def _host_routing(x, gate_w, e_bias):
    """fp32 numpy mirror of reference._routing_combine.

    Returns (emask [T,E] bool, combine [T,E] fp32)."""
    logits = x.astype(np.float32) @ gate_w.T.astype(np.float32)
    scores = 1.0 / (1.0 + np.exp(-logits))
    swb = scores + e_bias[None, :]
    swb_g = swb.reshape(T, G, E // G)
    gs = np.sort(swb_g, axis=-1)[..., -2:].sum(-1)          # top-2 sum per group
    thr4 = np.sort(gs, axis=-1)[:, -TOPK_GROUP][:, None]
    gmask = (gs >= thr4).astype(np.float32)
    smask = np.repeat(gmask, E // G, axis=-1)
    masked = swb * smask
    thr8 = np.sort(masked, axis=-1)[:, -TOP_K][:, None]
    emask = masked >= thr8
    sc = scores * emask
    combine = sc / (sc.sum(-1, keepdims=True) + 1e-20) * SCALE
    return emask, combine.astype(np.float32)


def _chunks(p, limit=512):
    """Split width p into chunks <= limit."""
    out = []
    o = 0
    while o < p:
        w = min(limit, p - o)
        out.append((o, w))
        o += w
    return out


def _make_tiers(counts):
    """Choose slot capacities + (expert, tok_offset, tok_len) piece assignment.

    Experts heavier than a threshold theta are split into near-equal pieces
    (duplicating that expert's weight DMA on the extra slots); pieces are
    sorted descending and striped into tiers of NCORES so the shared SPMD
    capacity of each tier hugs its pieces.  theta is picked by a small sweep
    minimizing max(PE-roofline, DMA-roofline)."""
    best = None
    for theta in (2048, 512, 420, 350, 300, 256):
        pieces = []
        for e in range(E):
            cnt = int(counts[e])
            if cnt == 0:
                continue
            m = max(1, -(-cnt // theta))
            base, rem = divmod(cnt, m)
            off = 0
            for j in range(m):
                ln = base + (1 if j < rem else 0)
                pieces.append((e, off, ln))
                off += ln
        S_ = -(-len(pieces) // NCORES)
        pieces.sort(key=lambda p: -p[2])
        pieces += [(-1, 0, 0)] * (S_ * NCORES - len(pieces))
        tiers = [pieces[k * NCORES:(k + 1) * NCORES] for k in range(S_)]
        P = [max(8, max(p[2] for p in tier)) for tier in tiers]
        pe_cyc = (sum(P) * 64 + sum(-(-pk // P128) for pk in P) * P128 * 32
                  + 49200)
        dma_b = len(P) * 3.15e6 + sum(P) * 2 * H + 3.7e6
        cost = max(pe_cyc / 2.4 * 1.06, dma_b / 360.0 + 6000)
        if best is None or cost < best[0]:
            best = (cost, tiers, P)
    return best[1], best[2]


def _pmajor(a, p=P128):
    """[k*128, n] -> contiguous [128, k, n] (partition-major for 1-shot DMA)."""
    k = a.shape[0] // p
    return np.ascontiguousarray(a.reshape(k, p, a.shape[1]).transpose(1, 0, 2))


def _iimajor(a):
    """[HT*128, IT*128] weight -> contiguous [128, IT, HT, 128] so each
    [:, ii] slice is one fully-contiguous DMA (per-ii streaming)."""
    m = a.shape[1] // P128
    b = a.reshape(HT, P128, m, P128).transpose(1, 2, 0, 3)
    return np.ascontiguousarray(b)


def _build_program(P):
    """Emit the SPMD Bass program for slot capacities P (list of SL ints)."""
    SL = len(P)
    nc = bacc.Bacc(target_bir_lowering=False, debug=False)

    # ---- DRAM parameters (per-core data arrives via in_maps) ----
    xe_d = [nc.dram_tensor(f"xe{k}", [P128, HT, P[k]], F16, kind="ExternalInput")
            for k in range(SL)]
    wg_d = [nc.dram_tensor(f"wg{k}", [P128, IT, HT, P128], F16,
                           kind="ExternalInput") for k in range(SL)]
    wu_d = [nc.dram_tensor(f"wu{k}", [P128, IT, HT, P128], F16,
                           kind="ExternalInput") for k in range(SL)]
    wd_d = [nc.dram_tensor(f"wd{k}", [P128, IT, H], F16, kind="ExternalInput")
            for k in range(SL)]
    nck = [(P[k] + P128 - 1) // P128 for k in range(SL)]
    sco = [sum(nck[:k]) for k in range(SL + 1)]
    sc_d = nc.dram_tensor("sc", [P128, sco[SL]], F32, kind="ExternalInput")
    xt_d = nc.dram_tensor("xt", [P128, T // 512, HT, 512], F16,
                          kind="ExternalInput")
    wsg_d = nc.dram_tensor("wsg", [P128, IST, HT, P128], F16,
                           kind="ExternalInput")
    wsu_d = nc.dram_tensor("wsu", [P128, IST, HT, P128], F16,
                           kind="ExternalInput")
    wsd_d = nc.dram_tensor("wsd", [P128, IST, H], F16, kind="ExternalInput")
    ro_d = [nc.dram_tensor(f"ro{k}", [P[k], H], F16, kind="ExternalOutput")
            for k in range(SL)]
    so_d = nc.dram_tensor("so", [T, H], F16, kind="ExternalOutput")

    with tile.TileContext(nc) as tc:
        with (
            tc.tile_pool(name="const", bufs=1) as cpool,
            tc.tile_pool(name="x", bufs=3) as xpool,
            tc.tile_pool(name="w", bufs=3) as wpool,
            tc.tile_pool(name="acts", bufs=2) as apool,
            tc.tile_pool(name="stage", bufs=3) as stpool,
            tc.tile_pool(name="ps", bufs=2, space="PSUM") as ps,
        ):
            # ---- loads, in consumption order ----
            sct = cpool.tile([P128, sco[SL]], F32, tag="sc")
            sc_t = [sct[:, sco[k]:sco[k + 1]] for k in range(SL)]

            xe_t, wg_t, wu_t, wd_t = {}, {}, {}, {}

            def _load_gu(k):
                xe_t[k] = xpool.tile([P128, HT, P[k]], F16, tag="xe", bufs=4,
                                     name=f"xe_t{k}")
                nc.sync.dma_start(out=xe_t[k][:], in_=xe_d[k][:])
                wg_t[k] = wpool.tile([P128, IT, HT, P128], F16, tag="wg",
                                     bufs=4, name=f"wg_t{k}")
                wu_t[k] = wpool.tile([P128, IT, HT, P128], F16, tag="wu",
                                     bufs=4, name=f"wu_t{k}")
                for ii in range(IT):
                    nc.sync.dma_start(out=wg_t[k][:, ii], in_=wg_d[k][:, ii])
                    nc.sync.dma_start(out=wu_t[k][:, ii], in_=wu_d[k][:, ii])

            def _load_wd(k):
                wd_t[k] = wpool.tile([P128, IT, H], F16, tag="wd", bufs=3,
                                     name=f"wd_t{k}")
                nc.sync.dma_start(out=wd_t[k][:], in_=wd_d[k][:])

            acts_t = {}

            def _gu_chain(k, mo, mw, ii):
                h1 = ps.tile([P128, 512], F32, tag="h1", name="h1")
                h2 = ps.tile([P128, 512], F32, tag="h2", name="h2")
                for h in range(HT):
                    nc.tensor.matmul(
                        h1[:, :mw], lhsT=wg_t[k][:, ii, h, :],
                        rhs=xe_t[k][:, h, mo:mo + mw],
                        start=(h == 0), stop=(h == HT - 1))
                for h in range(HT):
                    nc.tensor.matmul(
                        h2[:, :mw], lhsT=wu_t[k][:, ii, h, :],
                        rhs=xe_t[k][:, h, mo:mo + mw],
                        start=(h == 0), stop=(h == HT - 1))
                sl = stpool.tile([P128, 512], F32, tag="silu", bufs=3,
                                 name="sl")
                nc.scalar.activation(sl[:, :mw], h1[:, :mw], AF.Silu)
                nc.vector.tensor_mul(acts_t[k][ii][:, mo:mo + mw],
                                     sl[:, :mw], h2[:, :mw])

            def _slot_gu_chains(k):
                acts_t[k] = [apool.tile([P128, P[k]], F16, tag="acts", bufs=8,
                                        name=f"acts{ii}") for ii in range(IT)]
                return [(lambda k=k, mo=mo, mw=mw, ii=ii:
                         _gu_chain(k, mo, mw, ii))
                        for (mo, mw) in _chunks(P[k]) for ii in range(IT)]

            def _down_chain(k, cc):
                pp = min(P128, P[k] - cc * P128)
                ost = stpool.tile([P128, H], F16, tag="ost", bufs=4,
                                  name="ost")
                for hh in range(2):
                    dps = ps.tile([P128, H // 2], F32, tag="dps", bufs=4,
                                  name="dps")
                    for ii in range(IT):
                        nc.tensor.matmul(
                            dps[:pp, :],
                            lhsT=acts_t[k][ii][:, cc * P128:cc * P128 + pp],
                            rhs=wd_t[k][:, ii,
                                        hh * (H // 2):(hh + 1) * (H // 2)],
                            start=(ii == 0), stop=(ii == IT - 1))
                    nc.vector.tensor_scalar_mul(
                        ost[:pp, hh * (H // 2):(hh + 1) * (H // 2)],
                        dps[:pp, :], sc_t[k][:pp, cc:cc + 1])
                nc.gpsimd.dma_start(
                    out=ro_d[k][cc * P128:cc * P128 + pp, :],
                    in_=ost[:pp, :])

            def _slot_down_chains(k):
                return [(lambda k=k, cc=cc: _down_chain(k, cc))
                        for cc in range(nck[k])]

            def _merge(down, gu):
                """Emit down chains (short, DVE-paced) interleaved with the
                next phase's gate/up chains (long, PE-only): down leads, gu
                spread through the tail so the PE never idles on the DVE."""
                items = ([((i + 0.5) / len(down), f) for i, f in
                          enumerate(down)] +
                         [((j + 0.85) / len(gu), f) for j, f in
                          enumerate(gu)])
                for _, f in sorted(items, key=lambda x: x[0]):
                    f()

            # Load stream in consumption order; every tensor is ordered to
            # land a few us before its consuming phase reaches it.
            # first phase: a mid-sized slot whose compute covers the shared
            # loads; remaining slots descending with the smallest last (its
            # tiny final store shortens the drain)
            first = SL - 3
            rest = [k for k in range(SL) if k != first]

            _load_gu(first)
            _load_wd(first)
            nc.sync.dma_start(out=sct[:], in_=sc_d[:])

            xt_t = cpool.tile([P128, T // 512, HT, 512], F16, tag="xt")
            nc.sync.dma_start(out=xt_t[:, 0], in_=xt_d[:, 0])
            wsg_t = cpool.tile([P128, IST, HT, P128], F16, tag="wsg")
            wsu_t = cpool.tile([P128, IST, HT, P128], F16, tag="wsu")
            for ii in range(IST):
                nc.sync.dma_start(out=wsg_t[:, ii], in_=wsg_d[:, ii])
                nc.sync.dma_start(out=wsu_t[:, ii], in_=wsu_d[:, ii])
            nc.sync.dma_start(out=xt_t[:, 1], in_=xt_d[:, 1])
            wsd_t = cpool.tile([P128, IST, H], F16, tag="wsd")
            nc.sync.dma_start(out=wsd_t[:], in_=wsd_d[:])
            _load_gu(rest[0])
            _load_wd(rest[0])
            _load_gu(rest[1])
            _load_wd(rest[1])

            for f in _slot_gu_chains(first):
                f()
            for f in _slot_down_chains(first):
                f()

            # shared expert (intermediate shard ISH=256)
            acts_s = [apool.tile([P128, T], F16, tag="acts_s", bufs=2,
                                 name=f"acts_s{ii}") for ii in range(IST)]

            def _shared_gu_chain(mo, mw, ii):
                h1 = ps.tile([P128, 512], F32, tag="h1", name="h1")
                h2 = ps.tile([P128, 512], F32, tag="h2", name="h2")
                ci = mo // 512
                for h in range(HT):
                    nc.tensor.matmul(
                        h1[:, :mw], lhsT=wsg_t[:, ii, h, :],
                        rhs=xt_t[:, ci, h, :mw],
                        start=(h == 0), stop=(h == HT - 1))
                for h in range(HT):
                    nc.tensor.matmul(
                        h2[:, :mw], lhsT=wsu_t[:, ii, h, :],
                        rhs=xt_t[:, ci, h, :mw],
                        start=(h == 0), stop=(h == HT - 1))
                sl = stpool.tile([P128, 512], F32, tag="silu", bufs=3,
                                 name="sl")
                nc.scalar.activation(sl[:, :mw], h1[:, :mw], AF.Silu)
                nc.vector.tensor_mul(acts_s[ii][:, mo:mo + mw],
                                     sl[:, :mw], h2[:, :mw])

            for (mo, mw) in _chunks(T):
                for ii in range(IST):
                    _shared_gu_chain(mo, mw, ii)

            def _shared_down_chain(cc):
                ost = stpool.tile([P128, H], F16, tag="ost", bufs=4,
                                  name="ost")
                for hh in range(2):
                    dps = ps.tile([P128, H // 2], F32, tag="dps", bufs=4,
                                  name="dps")
                    for ii in range(IST):
                        nc.tensor.matmul(
                            dps[:],
                            lhsT=acts_s[ii][:, cc * P128:(cc + 1) * P128],
                            rhs=wsd_t[:, ii, hh * (H // 2):(hh + 1) * (H // 2)],
                            start=(ii == 0), stop=(ii == IST - 1))
                    nc.vector.tensor_copy(
                        ost[:, hh * (H // 2):(hh + 1) * (H // 2)], dps[:])
                nc.gpsimd.dma_start(
                    out=so_d[cc * P128:(cc + 1) * P128, :], in_=ost[:])

            # pipelined tail: down chains of each phase interleave with the
            # next slot's gate/up chains so the PE never idles on the DVE
            down_prev = [(lambda cc=cc: _shared_down_chain(cc))
                         for cc in range(T // P128)]
            for j, k in enumerate(rest):
                if j + 2 < len(rest):
                    _load_gu(rest[j + 2])
                    _load_wd(rest[j + 2])
                _merge(down_prev, _slot_gu_chains(k))
                down_prev = _slot_down_chains(k)
            for f in down_prev:
                f()

    nc.compile()
    return nc


def _prepare(inputs):
    """Host-side dispatch prep: returns (in_maps, P, slot_expert, tok_lists)."""
    x = np.ascontiguousarray(inputs["hidden_states"], dtype=np.float32)
    gate_w = np.asarray(inputs["gate_w"], dtype=np.float32)
    e_bias = np.asarray(inputs["e_bias"], dtype=np.float32)
    w_gate = np.asarray(inputs["w_gate"], dtype=np.float32)
    w_up = np.asarray(inputs["w_up"], dtype=np.float32)
    w_down = np.asarray(inputs["w_down"], dtype=np.float32)
    ws_gate = np.asarray(inputs["ws_gate"], dtype=np.float32)
    ws_up = np.asarray(inputs["ws_up"], dtype=np.float32)
    ws_down = np.asarray(inputs["ws_down"], dtype=np.float32)

    # ---- dispatch metadata ----
    emask, combine = _host_routing(x, gate_w, e_bias)
    counts = emask.sum(0).astype(np.int64)
    tok_lists = [np.nonzero(emask[:, e])[0] for e in range(E)]
    tiers, P = _make_tiers(counts)

    x16 = x.astype(np.float16)
    xtf = _pmajor(np.ascontiguousarray(x16.T))         # [128, HT, T]
    xt = np.ascontiguousarray(
        xtf.reshape(P128, HT, T // 512, 512).transpose(0, 2, 1, 3))
    in_maps = []
    slot_toks = []                                     # [core][slot] -> toks
    wg16 = w_gate.astype(np.float16)
    wu16 = w_up.astype(np.float16)
    wd16 = w_down.astype(np.float16)
    wgp = {}
    for c in range(NCORES):
        m = {"xt": xt,
             "wsg": _iimajor(ws_gate[:, c * ISH:(c + 1) * ISH].astype(np.float16)),
             "wsu": _iimajor(ws_up[:, c * ISH:(c + 1) * ISH].astype(np.float16)),
             "wsd": _pmajor(ws_down[c * ISH:(c + 1) * ISH, :].astype(np.float16))}
        st = []
        scs = []
        for k in range(len(P)):
            e, off, ln = tiers[k][c]
            toks = (tok_lists[e][off:off + ln] if e >= 0
                    else np.zeros(0, dtype=np.int64))
            st.append(toks)
            n = len(toks)
            n_c = (P[k] + P128 - 1) // P128
            xe = np.zeros((P128, HT, P[k]), dtype=np.float16)
            if n:
                xe[:, :, :n] = _pmajor(np.ascontiguousarray(x16[toks].T))
            scv = np.zeros(n_c * P128, dtype=np.float32)
            if n:
                scv[:n] = combine[toks, e]
            scs.append(np.ascontiguousarray(scv.reshape(n_c, P128).T))
            if e not in wgp:
                wgp[e] = ((_iimajor(wg16[e]), _iimajor(wu16[e]),
                           _pmajor(wd16[e])) if e >= 0 else
                          (np.zeros((P128, IT, HT, P128), np.float16),
                           np.zeros((P128, IT, HT, P128), np.float16),
                           np.zeros((P128, IT, H), np.float16)))
            m[f"xe{k}"] = xe
            m[f"wg{k}"], m[f"wu{k}"], m[f"wd{k}"] = wgp[e]
        m["sc"] = np.concatenate(scs, axis=1)
        slot_toks.append(st)
        in_maps.append(m)

    return in_maps, P, slot_toks


def _recombine(results, slot_toks):
    out = np.zeros((T, H), dtype=np.float32)
    for c in range(NCORES):
        out += results[c]["so"].astype(np.float32)
    for c in range(NCORES):
        for k, toks in enumerate(slot_toks[c]):
            if len(toks):
                out[toks] += results[c][f"ro{k}"][:len(toks)].astype(np.float32)
    return out


def kernel(**inputs):
    global LAST_RESULTS
    in_maps, P, slot_toks = _prepare(inputs)
    nc = _build_program(P)
    trace = bool(int(os.environ.get("KERNEL_TRACE", "0")))
    if trace:
        trace = _install_ntff_hook()
    LAST_RESULTS = run_bass_kernel_spmd(
        nc, in_maps, list(range(NCORES)), trace=trace)
    results = LAST_RESULTS.results
    return _recombine(results, slot_toks)
